# revision 22
# baseline (speedup 1.0000x reference)
"""Trainium2 Bass kernel for nn_Attention_module_52166672777937.

Data-parallel over batch across 8 NeuronCores (4 sequences per core),
with the 4 sequences x 8 heads STACKED on 32 partitions (s=(b,h)) so
every matmul serves all four sequences at once.

Algorithmic restructuring (validated vs the reference in bf16):
  * Only the LAST query row of causal attention is consumed, so scores
    are [32, L] per core, not [B,H,L,L].
  * x = emb[data] + pe is NEVER materialized.  Scores decompose as
      scores[s,l] = s_emb[s, data[l]] + (qk_s . peT[:,l]) + mask
    where s_emb = qkv @ emb.T is a per-head 256-entry lookup table and
    the data lookup is a one-hot matmul.
  * ctx = attn @ x @ Wv.T similarly decomposes:
      y = attn @ x = (attn @ onehot.T) @ emb + attn @ pe.
  * softmax normalization is folded into the attn transposes by using
    diag(1/denominator) as the transpose "identity" matrix.
  * q = Wq(emb[last] + pe[last]) + bq folds to  (emb@Wq.T).T @ onehot_last
    + qpe  with qpe computed host-side from lengths.
"""

import math
import sys

import ml_dtypes
import numpy as np

sys.path.insert(0, "/opt/trn_rl_repo")

import concourse.bacc as bacc
import concourse.bass as bass
import concourse.mybir as mybir
import concourse.tile as tile
from concourse.bass_utils import run_bass_kernel_spmd

dt = mybir.dt
AF = mybir.ActivationFunctionType
ALU = mybir.AluOpType
AX = mybir.AxisListType
PSUM = bass.MemorySpace.PSUM

N_CORES = 8
B, L = 32, 1000
LP = 1024
BPC = B // N_CORES        # 4 sequences per core
NS = BPC * 8              # 32 stacked (seq, head) rows
NCH = 256
E = 512
D = 512
NH, DH = 8, 64
HS = 512
NOUT = 8
SCALE = 1.0 / math.sqrt(DH)
NLC = LP // 128           # 8 position chunks


def _build():
    nc = bacc.Bacc(
        "TRN2", target_bir_lowering=False, debug=False, num_devices=N_CORES
    )

    f32 = dt.float32
    b16 = dt.bfloat16

    # ---- DRAM inputs -------------------------------------------------
    # f32 [128, 55]: qpe | b1 | b2(8 rows) | cvals | dT
    d_f32 = nc.dram_tensor("f32", [128, 91], f32, kind="ExternalInput")
    d_wqe = nc.dram_tensor("wqe", [NCH, D], b16, kind="ExternalInput")
    d_wk = nc.dram_tensor("wk", [D, E], b16, kind="ExternalInput")
    d_embT = nc.dram_tensor("embT", [E, NCH], b16, kind="ExternalInput")
    d_peT = nc.dram_tensor("peT", [E, LP], b16, kind="ExternalInput")
    # [4, 1056]: maskneg | E4 ; [32, 548]: hm32 | Rsel | id32
    d_m4 = nc.dram_tensor("m4", [BPC, 1056], b16, kind="ExternalInput")
    d_m32b = nc.dram_tensor("m32b", [NS, 548], b16, kind="ExternalInput")
    # [128, 288]: iotaC | hmask4
    d_sa = nc.dram_tensor("sa", [128, 256], b16, kind="ExternalInput")
    d_pe = nc.dram_tensor("pe", [LP, E], b16, kind="ExternalInput")
    d_emb = nc.dram_tensor("emb", [NCH, E], b16, kind="ExternalInput")
    d_wvT = nc.dram_tensor("wvT", [E, D], b16, kind="ExternalInput")
    d_w1T = nc.dram_tensor("w1T", [D, HS], b16, kind="ExternalInput")
    d_w2T = nc.dram_tensor("w2T", [HS, NOUT], b16, kind="ExternalInput")
    d_dbb = nc.dram_tensor("dbb", [128, BPC * LP], b16,
                           kind="ExternalInput")
    d_out = nc.dram_tensor("out", [1, BPC], f32, kind="ExternalOutput")

    with tile.TileContext(nc) as tc:
        with (
            tc.tile_pool(name="const", bufs=1) as cp,
            tc.tile_pool(name="work", bufs=2) as wp,
            tc.tile_pool(name="psbig", bufs=2, space=PSUM) as psb,
            tc.tile_pool(name="pstr", bufs=2, space=PSUM) as pst,
            tc.tile_pool(name="psw", bufs=2, space=PSUM) as psw,
            tc.tile_pool(name="psq", bufs=2, space=PSUM) as psq,
        ):
            # ------------- DMA: 3 queues, priority order ---------------
            # sync: f32 -> wqe -> peT halves -> m4 -> m32b
            f32_sb = cp.tile([128, 91], f32, name="f32", tag="f32")
            nc.sync.dma_start(out=f32_sb[:], in_=d_f32[:])
            qpe_sb = f32_sb[:, 0:16]     # [128, 4d x 4b]
            b1_sb = f32_sb[:, 16:20]
            b2_sb = f32_sb[0:NOUT, 20:21]
            cvals = f32_sb[:, 21:23]
            dTall = f32_sb[:, 23:55]
            hmask4 = f32_sb[:, 55:87]
            idxb_sb = f32_sb[:, 87:91]
            wqe_sb = cp.tile([128, 2, D], b16, name="wqe", tag="wqe")
            nc.sync.dma_start(
                out=wqe_sb[:], in_=d_wqe[:].rearrange("(c p) n -> p c n", p=128)
            )
            peT_sb = cp.tile([128, 4, LP], b16, name="peT", tag="peT")
            for hl in range(2):
                nc.sync.dma_start(
                    out=peT_sb[:, :, hl * 512:(hl + 1) * 512],
                    in_=d_peT[:, hl * 512:(hl + 1) * 512].rearrange(
                        "(c p) n -> p c n", p=128),
                )
            m4_sb = cp.tile([BPC, 1056], b16, name="m4", tag="m4")
            nc.sync.dma_start(out=m4_sb[:], in_=d_m4[:])
            maskneg = m4_sb[:, 0:LP]
            E4_sb = m4_sb[:, LP:LP + NS]
            m32b_sb = cp.tile([NS, 548], b16, name="m32b", tag="m32b")
            nc.sync.dma_start(out=m32b_sb[:], in_=d_m32b[:])
            hm32_sb = m32b_sb[:, 0:512]
            Rsel_sb = m32b_sb[:, 512:516]
            id32_sb = m32b_sb[:, 516:548]
            # bulk late-phase weights also on sync: the queue's outstanding
            # limit keeps them from competing with the critical transfers
            pe_sb = cp.tile([128, NLC, E], b16, name="pe", tag="pe")
            nc.sync.dma_start(
                out=pe_sb[:], in_=d_pe[:].rearrange("(c p) n -> p c n", p=128)
            )
            emb_sb = cp.tile([128, 2, E], b16, name="emb", tag="emb")
            nc.sync.dma_start(
                out=emb_sb[:], in_=d_emb[:].rearrange("(c p) n -> p c n", p=128)
            )
            wvT_sb = cp.tile([128, 4, D], b16, name="wvT", tag="wvT")
            nc.sync.dma_start(
                out=wvT_sb[:], in_=d_wvT[:].rearrange("(c p) n -> p c n", p=128)
            )
            w1T_sb = cp.tile([128, 4, HS], b16, name="w1T", tag="w1T")
            nc.sync.dma_start(
                out=w1T_sb[:], in_=d_w1T[:].rearrange("(c p) n -> p c n", p=128)
            )
            w2T_sb = cp.tile([128, 4, NOUT], b16, name="w2T", tag="w2T")
            nc.sync.dma_start(
                out=w2T_sb[:], in_=d_w2T[:].rearrange("(c p) n -> p c n", p=128)
            )
            # scalar: wk -> dbb0 -> embT -> dbb123 -> sa
            wk_sb = cp.tile([128, 4, E], b16, name="wk", tag="wk")
            nc.scalar.dma_start(
                out=wk_sb[:], in_=d_wk[:].rearrange("(c p) n -> p c n", p=128)
            )
            dbb_sb = cp.tile([128, BPC, LP], b16, name="dbb", tag="dbb")
            nc.scalar.dma_start(out=dbb_sb[:, 0, :], in_=d_dbb[:, 0:LP])
            embT_sb = cp.tile([128, 4, NCH], b16, name="embT", tag="embT")
            nc.scalar.dma_start(
                out=embT_sb[:],
                in_=d_embT[:].rearrange("(c p) n -> p c n", p=128),
            )
            for b in range(1, BPC):
                nc.scalar.dma_start(
                    out=dbb_sb[:, b, :],
                    in_=d_dbb[:, b * LP:(b + 1) * LP],
                )
            sa_sb = cp.tile([128, 256], b16, name="sa", tag="sa")
            nc.scalar.dma_start(out=sa_sb[:], in_=d_sa[:])
            iotaC = sa_sb[:, 0:256]
            ones8_sb = cp.tile([NOUT, 1], b16, name="ones8", tag="ones8")
            nc.gpsimd.memset(ones8_sb[:], 1.0)
            s_embm = cp.tile([128, 2, BPC, NS], b16, name="s_embm",
                             tag="s_embm")
            nc.gpsimd.memset(s_embm[:], 0.0)

            # ------------- q prep (emitted first: heads the queues) ---
            ohl = cp.tile([128, 2, BPC], b16, name="ohl", tag="ohl")
            for c in range(2):
                nc.vector.tensor_scalar(
                    ohl[:, c, :], idxb_sb, cvals[:, c:c + 1], None,
                    ALU.is_equal,
                )
            # qT [d, b] = WqE.T @ ohl + qpe   (WqE = emb @ Wq.T)
            qT_sb = cp.tile([128, 4, BPC], f32, name="qT", tag="qT")
            for dch in range(4):
                p = psq.tile([128, BPC], f32, name=f"qp{dch}", tag="qp")
                for c in range(2):
                    nc.tensor.matmul(
                        p[:], wqe_sb[:, c, dch * 128:(dch + 1) * 128],
                        ohl[:, c, :], start=(c == 0), stop=(c == 1),
                    )
                nc.vector.tensor_tensor(
                    qT_sb[:, dch, :], p[:],
                    qpe_sb[:, dch * BPC:(dch + 1) * BPC], ALU.add,
                )
            # qblk [d, s] = q[d, b] * hmask[d, h]
            qblk = cp.tile([128, 4, NS], b16, name="qblk", tag="qblk")
            for dch in range(4):
                eng = nc.vector if dch < 2 else nc.gpsimd
                for b in range(BPC):
                    eng.tensor_scalar(
                        qblk[:, dch, b * NH:(b + 1) * NH],
                        hmask4[:, dch * NH:(dch + 1) * NH],
                        qT_sb[:, dch, b:b + 1], None, ALU.mult,
                    )
            # qkvT [e, s] = Wk.T @ qblk, scaled by 1/sqrt(dh)
            qkvT = cp.tile([128, 4, NS], b16, name="qkvT", tag="qkvT")
            for ech in range(4):
                p = psq.tile([128, NS], f32, name=f"qkp{ech}", tag="qp")
                for dch in range(4):
                    nc.tensor.matmul(
                        p[:], wk_sb[:, dch, ech * 128:(ech + 1) * 128],
                        qblk[:, dch, :], start=(dch == 0), stop=(dch == 3),
                    )
                nc.vector.tensor_scalar(
                    qkvT[:, ech, :], p[:], SCALE, None, ALU.mult,
                )
            # s_embT [c, s] = embT.T @ qkvT, evicted per-seq masked
            for c in range(2):
                p = psq.tile([128, NS], f32, name=f"sep{c}", tag="qp")
                for ech in range(4):
                    nc.tensor.matmul(
                        p[:], embT_sb[:, ech, c * 128:(c + 1) * 128],
                        qkvT[:, ech, :], start=(ech == 0), stop=(ech == 3),
                    )
                for b in range(BPC):
                    nc.scalar.copy(
                        s_embm[:, c, b, b * NH:(b + 1) * NH],
                        p[:, b * NH:(b + 1) * NH],
                    )

            # ------------- one-hots [c, l] on DVE ---------------------
            oh_sb = cp.tile([128, 2, BPC, LP], b16, name="oh", tag="oh")
            for b in range(BPC):
                for c in range(2):
                    eng = nc.gpsimd if b == 3 else nc.vector
                    eng.tensor_scalar(
                        oh_sb[:, c, b, :], dbb_sb[:, b, :], cvals[:, c:c + 1],
                        None, ALU.is_equal,
                    )

            # ------------- scores [32, L] + exp -----------------------
            attn = cp.tile([NS, LP], b16, name="attn", tag="attn")
            dnh = wp.tile([NS, 2], f32, name="dnh", tag="dnh")
            for hl in range(2):
                sc = psb.tile([NS, 512], f32, name=f"sc{hl}", tag="big")
                lo, hi = hl * 512, (hl + 1) * 512
                for ech in range(4):
                    nc.tensor.matmul(
                        sc[:], qkvT[:, ech, :], peT_sb[:, ech, lo:hi],
                        start=(ech == 0), stop=False,
                    )
                for b in range(BPC):
                    for c in range(2):
                        nc.tensor.matmul(
                            sc[:], s_embm[:, c, b, :], oh_sb[:, c, b, lo:hi],
                            start=False, stop=False,
                        )
                nc.tensor.matmul(
                    sc[:], E4_sb, maskneg[:, lo:hi], start=False, stop=True,
                )
                nc.scalar.activation(attn[:, lo:hi], sc[:], AF.Exp,
                                     accum_out=dnh[:, hl:hl + 1])

            # ------------- one-hots [l, c] (first half) ---------------
            # ohT layout: [l, pair, lc, j, c] so each w matmul streams the
            # two sequences of a pair as one 512-wide moving operand
            ohT_sb = cp.tile([128, 2, NLC, 2, NCH], b16, name="ohT",
                             tag="ohT")
            for b in range(2):
                for lc in range(NLC):
                    nc.vector.tensor_scalar(
                        ohT_sb[:, b // 2, lc, b % 2, :], iotaC,
                        dTall[:, b * NLC + lc:b * NLC + lc + 1], None,
                        ALU.is_equal,
                    )

            for b in range(2, BPC):
                for lc in range(NLC):
                    nc.vector.tensor_scalar(
                        ohT_sb[:, b // 2, lc, b % 2, :], iotaC,
                        dTall[:, b * NLC + lc:b * NLC + lc + 1], None,
                        ALU.is_equal,
                    )

            # ------------- aT [l, s] = attn.T (unnormalized) ----------
            aT = cp.tile([128, NLC, NS], b16, name="aT", tag="aT")
            for lc in range(NLC):
                tp = pst.tile([128, NS], b16, name=f"tp{lc}", tag="tr",
                              bufs=2)
                nc.tensor.transpose(
                    tp[:], attn[:, lc * 128:(lc + 1) * 128], id32_sb
                )
                if lc % 2 == 0:
                    nc.vector.tensor_copy(aT[:, lc, :], tp[:])
                else:
                    nc.scalar.copy(aT[:, lc, :], tp[:])
            # softmax denominator (only the y eviction consumes it)
            dn = wp.tile([NS, 1], f32, name="dn", tag="dn")
            nc.vector.tensor_tensor(dn[:], dnh[:, 0:1], dnh[:, 1:2], ALU.add)
            rec = wp.tile([NS, 1], f32, name="rec", tag="rec")
            nc.vector.reciprocal(rec[:], dn[:])

            # ------------- y_pe, then per-seq w -> wT (pipelined) -----
            yp = psb.tile([NS, E], f32, name="yp", tag="big")
            for lc in range(NLC):
                nc.tensor.matmul(
                    yp[:], aT[:, lc, :], pe_sb[:, lc, :],
                    start=(lc == 0), stop=False,
                )
            w32_sb = cp.tile([NS, 2, 512], b16, name="w32", tag="w32")
            wT_all = cp.tile([128, 2, NS], b16, name="wT_all", tag="wT_all")
            for pr in range(2):
                wpp = psw.tile([NS, 512], f32, name=f"wp{pr}", tag="wp")
                for lc in range(NLC):
                    nc.tensor.matmul(
                        wpp[:], aT[:, lc, :], ohT_sb[:, pr, lc, :, :],
                        start=(lc == 0), stop=(lc == NLC - 1),
                    )
                for cc in range(4):
                    sl_ = slice(cc * 128, (cc + 1) * 128)
                    if cc % 2 == 0:
                        nc.vector.tensor_copy(w32_sb[:, pr, sl_], wpp[:, sl_])
                    else:
                        nc.scalar.copy(w32_sb[:, pr, sl_], wpp[:, sl_])
                for cc in range(4):
                    b = pr * 2 + cc // 2
                    c = cc % 2
                    tp = pst.tile([128, NS], b16, name=f"wt{pr}_{cc}",
                                  tag="tr", bufs=2)
                    nc.tensor.transpose(
                        tp[:], w32_sb[:, pr, cc * 128:(cc + 1) * 128],
                        id32_sb,
                    )
                    if c == 0:
                        nc.vector.tensor_copy(
                            wT_all[:, c, b * NH:(b + 1) * NH],
                            tp[:, b * NH:(b + 1) * NH])
                    else:
                        nc.scalar.copy(
                            wT_all[:, c, b * NH:(b + 1) * NH],
                            tp[:, b * NH:(b + 1) * NH])

            # ------------- y += wT.T @ emb ; yT ; z = y @ Wv.T --------
            for c in range(2):
                nc.tensor.matmul(
                    yp[:], wT_all[:, c, :], emb_sb[:, c, :],
                    start=False, stop=(c == 1),
                )
            y_sb = wp.tile([NS, E], b16, name="y_sb", tag="y_sb")
            for k in range(4):
                sl_ = slice(k * 128, (k + 1) * 128)
                if k % 2 == 0:
                    nc.vector.tensor_scalar(y_sb[:, sl_], yp[:, sl_],
                                            rec[:], None, ALU.mult)
                else:
                    nc.scalar.activation(y_sb[:, sl_], yp[:, sl_], AF.Copy,
                                         scale=rec[:])
            yT = cp.tile([128, 4, NS], b16, name="yT", tag="yT")
            zp = psb.tile([NS, D], f32, name="zp", tag="big")
            for ech in range(4):
                tp = pst.tile([128, NS], b16, name=f"yt{ech}", tag="tr",
                              bufs=2)
                nc.tensor.transpose(
                    tp[:], y_sb[:, ech * 128:(ech + 1) * 128], id32_sb
                )
                if ech % 2 == 0:
                    nc.vector.tensor_copy(yT[:, ech, :], tp[:])
                else:
                    nc.scalar.copy(yT[:, ech, :], tp[:])
            for ech in range(4):
                nc.tensor.matmul(
                    zp[:], yT[:, ech, :], wvT_sb[:, ech, :],
                    start=(ech == 0), stop=(ech == 3),
                )
            zm = wp.tile([NS, D], b16, name="zm", tag="zm")
            for zh in range(2):
                nc.vector.tensor_tensor(
                    zm[:, zh * 256:(zh + 1) * 256],
                    zp[:, zh * 256:(zh + 1) * 256],
                    hm32_sb[:, zh * 256:(zh + 1) * 256], ALU.mult)
            ctxT = cp.tile([128, 4, BPC], b16, name="ctxT", tag="ctxT")
            for m in range(4):
                p = pst.tile([128, BPC], f32, name=f"cx{m}", tag="tr",
                             bufs=2)
                nc.tensor.matmul(
                    p[:], zm[:, m * 128:(m + 1) * 128], Rsel_sb,
                )
                if m % 2 == 0:
                    nc.vector.tensor_copy(ctxT[:, m, :], p[:])
                else:
                    nc.scalar.copy(ctxT[:, m, :], p[:])

            # ------------- prediction head ----------------------------
            hT = cp.tile([128, 4, BPC], b16, name="hT", tag="hT")
            for hc in range(4):
                p = psq.tile([128, BPC], f32, name=f"hp{hc}", tag="qp")
                for m in range(4):
                    nc.tensor.matmul(
                        p[:], w1T_sb[:, m, hc * 128:(hc + 1) * 128],
                        ctxT[:, m, :], start=(m == 0), stop=(m == 3),
                    )
                t1 = wp.tile([128, BPC], f32, name=f"t1_{hc}", tag="t1",
                             bufs=2)
                nc.vector.tensor_scalar(t1[:], p[:], b1_sb[:, hc:hc + 1],
                                        None, ALU.add)
                nc.vector.scalar_tensor_tensor(
                    hT[:, hc, :], t1[:], 0.01, t1[:], ALU.mult, ALU.max
                )
            r2p = pst.tile([NOUT, BPC], f32, name="r2p", tag="tr", bufs=2)
            for hc in range(4):
                nc.tensor.matmul(
                    r2p[:], w2T_sb[:, hc, :], hT[:, hc, :],
                    start=(hc == 0), stop=(hc == 3),
                )
            r_sb = wp.tile([NOUT, BPC], b16, name="r_sb", tag="r_sb")
            nc.vector.tensor_scalar(r_sb[:], r2p[:], b2_sb, 0.0,
                                    ALU.add, ALU.max)
            mp = pst.tile([1, BPC], f32, name="mp", tag="tr", bufs=2)
            nc.tensor.matmul(mp[:], ones8_sb[:], r_sb[:])
            mt = wp.tile([1, BPC], f32, name="mt", tag="mt")
            nc.vector.tensor_scalar(mt[:], mp[:], 1.0 / NOUT, None, ALU.mult)
            out_sb = cp.tile([1, BPC], f32, name="out_sb", tag="out_sb")
            nc.vector.scalar_tensor_tensor(
                out_sb[:], mt[:], 0.01, mt[:], ALU.mult, ALU.max
            )
            nc.sync.dma_start(out=d_out[:], in_=out_sb[:])

    nc.compile()
    return nc


_CACHE = {}


def _get_module():
    if "nc" not in _CACHE:
        _CACHE["nc"] = _build()
    return _CACHE["nc"]


def _pos_encoding():
    pos = np.arange(LP, dtype=np.float32)[:, None]
    div = np.exp(
        np.arange(0, D, 2, dtype=np.float32) * (-math.log(10000.0) / D)
    )
    pe = np.zeros((LP, D), np.float32)
    pe[:, 0::2] = np.sin(pos * div)
    pe[:, 1::2] = np.cos(pos * div)
    return pe


def make_in_maps(data, lengths, emb, Wq, bq, Wk, bk, Wv, bv, W1, b1, W2, b2):
    # the kernel folds the K-projection into the score lookup; a nonzero
    # bk would add a per-head constant to the scores (bk is zero here).
    assert float(np.abs(np.asarray(bk)).max()) == 0.0
    assert float(np.abs(np.asarray(bv)).max()) == 0.0

    b16 = ml_dtypes.bfloat16
    emb = np.asarray(emb, np.float32)
    Wq, Wk, Wv = (np.asarray(a, np.float32) for a in (Wq, Wk, Wv))
    W1, W2 = np.asarray(W1, np.float32), np.asarray(W2, np.float32)
    pe = _pos_encoding()                          # [LP, D]
    data = np.asarray(data)
    lengths = np.asarray(lengths)
    p = (lengths.astype(np.int64) - 1)

    WqE = emb @ Wq.T                              # [256, 512]
    qpe_full = Wq @ pe[p].T + np.asarray(bq, np.float32)[:, None]  # [D, B]
    hmask = np.repeat(np.eye(NH, dtype=np.float32), DH, axis=0)    # [D, 8]

    cvals = (np.arange(2)[None, :] * 128
             + np.arange(128)[:, None]).astype(np.float32)
    iotaC = np.broadcast_to(np.arange(NCH, dtype=np.float32), (128, NCH))
    hmask4 = hmask.reshape(4, 128, NH).transpose(1, 0, 2).reshape(128, 32)

    E4 = np.zeros((BPC, NS), np.float32)
    for b in range(BPC):
        E4[b, b * NH:(b + 1) * NH] = 1.0
    hm32 = np.zeros((NS, D), np.float32)
    for b in range(BPC):
        for h in range(NH):
            hm32[b * NH + h, h * DH:(h + 1) * DH] = 1.0
    Rsel = np.zeros((NS, BPC), np.float32)
    for b in range(BPC):
        Rsel[b * NH:(b + 1) * NH, b] = 1.0
    id32 = np.eye(NS, dtype=np.float32)

    dpad = np.zeros((B, LP), np.int64)
    dpad[:, :L] = data
    idxl = data[np.arange(B), p].astype(np.float32)

    shared = {
        "wqe": np.ascontiguousarray(WqE, dtype=b16),
        "wk": np.ascontiguousarray(Wk, dtype=b16),
        "embT": np.ascontiguousarray(emb.T, dtype=b16),
        "peT": np.ascontiguousarray(pe.T, dtype=b16),
        "pe": np.ascontiguousarray(pe, dtype=b16),
        "emb": np.ascontiguousarray(emb, dtype=b16),
        "wvT": np.ascontiguousarray(Wv.T, dtype=b16),
        "w1T": np.ascontiguousarray(W1.T, dtype=b16),
        "w2T": np.ascontiguousarray(W2.T, dtype=b16),
    }
    shared["sa"] = np.ascontiguousarray(iotaC, dtype=b16)

    in_maps = []
    for core in range(N_CORES):
        sl = slice(core * BPC, (core + 1) * BPC)
        m = dict(shared)
        dT = np.zeros((128, 32), np.float32)
        for b in range(BPC):
            for lc in range(NLC):
                dT[:, b * NLC + lc] = dpad[sl][b, lc * 128:(lc + 1) * 128]

        maskneg = np.where(
            np.arange(LP)[None, :] > p[sl][:, None], -30000.0, 0.0
        ).astype(np.float32)                       # [4, LP]
        m["m4"] = np.ascontiguousarray(
            np.concatenate([maskneg, E4], axis=1), dtype=b16)
        m["m32b"] = np.ascontiguousarray(
            np.concatenate([hm32, Rsel, id32], axis=1), dtype=b16)

        fb = np.zeros((128, 91), np.float32)
        fb[:, 0:16] = qpe_full[:, sl].reshape(4, 128, BPC).transpose(
            1, 0, 2).reshape(128, 16)
        fb[:, 16:20] = np.asarray(b1, np.float32).reshape(4, 128).T
        fb[0:NOUT, 20] = np.asarray(b2, np.float32)
        fb[:, 21:23] = cvals
        fb[:, 23:55] = dT
        fb[:, 55:87] = hmask4
        fb[:, 87:91] = np.broadcast_to(idxl[sl], (128, BPC))
        m["f32"] = np.ascontiguousarray(fb)

        m["dbb"] = np.ascontiguousarray(np.broadcast_to(
            dpad[sl].astype(np.float32).reshape(1, -1), (128, BPC * LP)
        ).astype(b16))
        in_maps.append(m)
    return in_maps


def kernel(data, lengths, emb, Wq, bq, Wk, bk, Wv, bv, W1, b1, W2, b2):
    nc = _get_module()
    in_maps = make_in_maps(
        np.asarray(data), np.asarray(lengths), emb, Wq, bq, Wk, bk, Wv, bv,
        W1, b1, W2, b2,
    )
    res = run_bass_kernel_spmd(nc, in_maps, list(range(N_CORES)))
    out = np.concatenate(
        [res.results[c]["out"].reshape(BPC) for c in range(N_CORES)]
    )
    return out.astype(np.float32)


# revision 23
# speedup vs baseline: 1.5689x; 1.5689x over previous
"""Trainium2 Bass kernel for nn_Attention_module_52166672777937.

Data-parallel over batch across 8 NeuronCores (4 sequences per core),
with the 4 sequences x 8 heads STACKED on 32 partitions (s=(b,h)) so
every matmul serves all four sequences at once.

Algorithmic restructuring (validated vs the reference in bf16):
  * Only the LAST query row of causal attention is consumed, so scores
    are [32, L] per core, not [B,H,L,L].
  * x = emb[data] + pe is NEVER materialized.  Scores decompose as
      scores[s,l] = s_emb[s, data[l]] + (qk_s . peT[:,l]) + mask
    where s_emb = qkv @ emb.T is a per-head 256-entry lookup table and
    the data lookup is a one-hot matmul.
  * ctx = attn @ x @ Wv.T similarly decomposes:
      y = attn @ x = (attn @ onehot.T) @ emb + attn @ pe.
  * softmax normalization is folded into the attn transposes by using
    diag(1/denominator) as the transpose "identity" matrix.
  * q = Wq(emb[last] + pe[last]) + bq folds to  (emb@Wq.T).T @ onehot_last
    + qpe  with qpe computed host-side from lengths.
"""

import math
import sys

import ml_dtypes
import numpy as np

sys.path.insert(0, "/opt/trn_rl_repo")

import concourse.bacc as bacc
import concourse.bass as bass
import concourse.mybir as mybir
import concourse.tile as tile
from concourse.bass_utils import run_bass_kernel_spmd

dt = mybir.dt
AF = mybir.ActivationFunctionType
ALU = mybir.AluOpType
AX = mybir.AxisListType
PSUM = bass.MemorySpace.PSUM

N_CORES = 8
B, L = 32, 1000
LP = 1024
BPC = B // N_CORES        # 4 sequences per core
NS = BPC * 8              # 32 stacked (seq, head) rows
NCH = 256
E = 512
D = 512
NH, DH = 8, 64
HS = 512
NOUT = 8
SCALE = 1.0 / math.sqrt(DH)
NLC = LP // 128           # 8 position chunks


def _build():
    nc = bacc.Bacc(
        "TRN2", target_bir_lowering=False, debug=False, num_devices=N_CORES
    )

    f32 = dt.float32
    b16 = dt.bfloat16

    # ---- DRAM inputs -------------------------------------------------
    # f32 [128, 55]: qpe | b1 | b2(8 rows) | cvals | dT
    d_f32 = nc.dram_tensor("f32", [128, 91], f32, kind="ExternalInput")
    d_wqe = nc.dram_tensor("wqe", [NCH, D], b16, kind="ExternalInput")
    d_wk = nc.dram_tensor("wk", [D, E], b16, kind="ExternalInput")
    d_embT = nc.dram_tensor("embT", [E, NCH], b16, kind="ExternalInput")
    d_peT = nc.dram_tensor("peT", [E, LP], b16, kind="ExternalInput")
    # [4, 1056]: maskneg | E4 ; [32, 548]: hm32 | Rsel | id32
    d_m4 = nc.dram_tensor("m4", [BPC, 1056], b16, kind="ExternalInput")
    d_m32b = nc.dram_tensor("m32b", [NS, 548], b16, kind="ExternalInput")
    # [128, 288]: iotaC | hmask4
    d_sa = nc.dram_tensor("sa", [128, 256], b16, kind="ExternalInput")
    d_pe = nc.dram_tensor("pe", [LP, E], b16, kind="ExternalInput")
    d_emb = nc.dram_tensor("emb", [NCH, E], b16, kind="ExternalInput")
    d_wvT = nc.dram_tensor("wvT", [E, D], b16, kind="ExternalInput")
    d_w1T = nc.dram_tensor("w1T", [D, HS], b16, kind="ExternalInput")
    d_w2T = nc.dram_tensor("w2T", [HS, NOUT], b16, kind="ExternalInput")
    d_dbb = nc.dram_tensor("dbb", [128, BPC * LP], b16,
                           kind="ExternalInput")
    d_out = nc.dram_tensor("out", [1, BPC], f32, kind="ExternalOutput")

    with tile.TileContext(nc) as tc:
        with (
            tc.tile_pool(name="const", bufs=1) as cp,
            tc.tile_pool(name="work", bufs=2) as wp,
            tc.tile_pool(name="psbig", bufs=2, space=PSUM) as psb,
            tc.tile_pool(name="pstr", bufs=2, space=PSUM) as pst,
            tc.tile_pool(name="psw", bufs=2, space=PSUM) as psw,
            tc.tile_pool(name="psq", bufs=2, space=PSUM) as psq,
        ):
            # ------------- DMA: 3 queues, priority order ---------------
            # sync: f32 -> wqe -> peT halves -> m4 -> m32b
            f32_sb = cp.tile([128, 91], f32, name="f32", tag="f32")
            nc.sync.dma_start(out=f32_sb[:], in_=d_f32[:])
            qpe_sb = f32_sb[:, 0:16]     # [128, 4d x 4b]
            b1_sb = f32_sb[:, 16:20]
            b2_sb = f32_sb[0:NOUT, 20:21]
            cvals = f32_sb[:, 21:23]
            dTall = f32_sb[:, 23:55]
            hmask4 = f32_sb[:, 55:87]
            idxb_sb = f32_sb[:, 87:91]
            wqe_sb = cp.tile([128, 2, D], b16, name="wqe", tag="wqe")
            nc.sync.dma_start(
                out=wqe_sb[:], in_=d_wqe[:].rearrange("(c p) n -> p c n", p=128)
            )
            peT_sb = cp.tile([128, 4, LP], b16, name="peT", tag="peT")
            for hl in range(2):
                nc.sync.dma_start(
                    out=peT_sb[:, :, hl * 512:(hl + 1) * 512],
                    in_=d_peT[:, hl * 512:(hl + 1) * 512].rearrange(
                        "(c p) n -> p c n", p=128),
                )
            m4_sb = cp.tile([BPC, 1056], b16, name="m4", tag="m4")
            nc.sync.dma_start(out=m4_sb[:], in_=d_m4[:])
            maskneg = m4_sb[:, 0:LP]
            E4_sb = m4_sb[:, LP:LP + NS]
            m32b_sb = cp.tile([NS, 548], b16, name="m32b", tag="m32b")
            nc.sync.dma_start(out=m32b_sb[:], in_=d_m32b[:])
            hm32_sb = m32b_sb[:, 0:512]
            Rsel_sb = m32b_sb[:, 512:516]
            id32_sb = m32b_sb[:, 516:548]
            # bulk late-phase weights also on sync: the queue's outstanding
            # limit keeps them from competing with the critical transfers
            pe_sb = cp.tile([128, NLC, E], b16, name="pe", tag="pe")
            nc.sync.dma_start(
                out=pe_sb[:], in_=d_pe[:].rearrange("(c p) n -> p c n", p=128)
            )
            emb_sb = cp.tile([128, 2, E], b16, name="emb", tag="emb")
            nc.sync.dma_start(
                out=emb_sb[:], in_=d_emb[:].rearrange("(c p) n -> p c n", p=128)
            )
            wvT_sb = cp.tile([128, 4, D], b16, name="wvT", tag="wvT")
            nc.sync.dma_start(
                out=wvT_sb[:], in_=d_wvT[:].rearrange("(c p) n -> p c n", p=128)
            )
            w1T_sb = cp.tile([128, 4, HS], b16, name="w1T", tag="w1T")
            nc.sync.dma_start(
                out=w1T_sb[:], in_=d_w1T[:].rearrange("(c p) n -> p c n", p=128)
            )
            w2T_sb = cp.tile([128, 4, NOUT], b16, name="w2T", tag="w2T")
            nc.sync.dma_start(
                out=w2T_sb[:], in_=d_w2T[:].rearrange("(c p) n -> p c n", p=128)
            )
            # scalar: wk -> dbb0 -> embT -> dbb123 -> sa
            wk_sb = cp.tile([128, 4, E], b16, name="wk", tag="wk")
            nc.scalar.dma_start(
                out=wk_sb[:], in_=d_wk[:].rearrange("(c p) n -> p c n", p=128)
            )
            dbb_sb = cp.tile([128, BPC, LP], b16, name="dbb", tag="dbb")
            nc.scalar.dma_start(out=dbb_sb[:, 0, :], in_=d_dbb[:, 0:LP])
            embT_sb = cp.tile([128, 4, NCH], b16, name="embT", tag="embT")
            nc.scalar.dma_start(
                out=embT_sb[:],
                in_=d_embT[:].rearrange("(c p) n -> p c n", p=128),
            )
            for b in range(1, BPC):
                nc.scalar.dma_start(
                    out=dbb_sb[:, b, :],
                    in_=d_dbb[:, b * LP:(b + 1) * LP],
                )
            sa_sb = cp.tile([128, 256], b16, name="sa", tag="sa")
            nc.scalar.dma_start(out=sa_sb[:], in_=d_sa[:])
            iotaC = sa_sb[:, 0:256]
            ones8_sb = cp.tile([NOUT, 1], b16, name="ones8", tag="ones8")
            nc.gpsimd.memset(ones8_sb[:], 1.0)
            s_embm = cp.tile([128, 2, BPC, NS], b16, name="s_embm",
                             tag="s_embm")
            nc.gpsimd.memset(s_embm[:], 0.0)

            # ------------- q prep (emitted first: heads the queues) ---
            ohl = cp.tile([128, 2, BPC], b16, name="ohl", tag="ohl")
            for c in range(2):
                nc.vector.tensor_scalar(
                    ohl[:, c, :], idxb_sb, cvals[:, c:c + 1], None,
                    ALU.is_equal,
                )
            # qT [d, b] = WqE.T @ ohl + qpe   (WqE = emb @ Wq.T)
            qT_sb = cp.tile([128, 4, BPC], f32, name="qT", tag="qT")
            for dch in range(4):
                p = psq.tile([128, BPC], f32, name=f"qp{dch}", tag="qp")
                for c in range(2):
                    nc.tensor.matmul(
                        p[:], wqe_sb[:, c, dch * 128:(dch + 1) * 128],
                        ohl[:, c, :], start=(c == 0), stop=(c == 1),
                    )
                nc.vector.tensor_tensor(
                    qT_sb[:, dch, :], p[:],
                    qpe_sb[:, dch * BPC:(dch + 1) * BPC], ALU.add,
                )
            # qblk [d, s] = q[d, b] * hmask[d, h]
            qblk = cp.tile([128, 4, NS], b16, name="qblk", tag="qblk")
            for dch in range(4):
                eng = nc.vector if dch < 2 else nc.gpsimd
                for b in range(BPC):
                    eng.tensor_scalar(
                        qblk[:, dch, b * NH:(b + 1) * NH],
                        hmask4[:, dch * NH:(dch + 1) * NH],
                        qT_sb[:, dch, b:b + 1], None, ALU.mult,
                    )
            # qkvT [e, s] = Wk.T @ qblk, scaled by 1/sqrt(dh)
            qkvT = cp.tile([128, 4, NS], b16, name="qkvT", tag="qkvT")
            for ech in range(4):
                p = psq.tile([128, NS], f32, name=f"qkp{ech}", tag="qp")
                for dch in range(4):
                    nc.tensor.matmul(
                        p[:], wk_sb[:, dch, ech * 128:(ech + 1) * 128],
                        qblk[:, dch, :], start=(dch == 0), stop=(dch == 3),
                    )
                nc.vector.tensor_scalar(
                    qkvT[:, ech, :], p[:], SCALE, None, ALU.mult,
                )
            # s_embT [c, s] = embT.T @ qkvT, evicted per-seq masked
            for c in range(2):
                p = psq.tile([128, NS], f32, name=f"sep{c}", tag="qp")
                for ech in range(4):
                    nc.tensor.matmul(
                        p[:], embT_sb[:, ech, c * 128:(c + 1) * 128],
                        qkvT[:, ech, :], start=(ech == 0), stop=(ech == 3),
                    )
                for b in range(BPC):
                    nc.scalar.copy(
                        s_embm[:, c, b, b * NH:(b + 1) * NH],
                        p[:, b * NH:(b + 1) * NH],
                    )

            # ------------- one-hots [c, l] on DVE ---------------------
            oh_sb = cp.tile([128, 2, BPC, LP], b16, name="oh", tag="oh")
            for b in range(BPC):
                for c in range(2):
                    nc.vector.tensor_scalar(
                        oh_sb[:, c, b, :], dbb_sb[:, b, :], cvals[:, c:c + 1],
                        None, ALU.is_equal,
                    )

            # ------------- scores [32, L] + exp -----------------------
            attn = cp.tile([NS, LP], b16, name="attn", tag="attn")
            dnh = wp.tile([NS, 2], f32, name="dnh", tag="dnh")
            for hl in range(2):
                sc = psb.tile([NS, 512], f32, name=f"sc{hl}", tag="big")
                lo, hi = hl * 512, (hl + 1) * 512
                for ech in range(4):
                    nc.tensor.matmul(
                        sc[:], qkvT[:, ech, :], peT_sb[:, ech, lo:hi],
                        start=(ech == 0), stop=False,
                    )
                for b in range(BPC):
                    for c in range(2):
                        nc.tensor.matmul(
                            sc[:], s_embm[:, c, b, :], oh_sb[:, c, b, lo:hi],
                            start=False, stop=False,
                        )
                nc.tensor.matmul(
                    sc[:], E4_sb, maskneg[:, lo:hi], start=False, stop=True,
                )
                nc.scalar.activation(attn[:, lo:hi], sc[:], AF.Exp,
                                     accum_out=dnh[:, hl:hl + 1])

            # ------------- one-hots [l, c] (first half) ---------------
            # ohT layout: [l, pair, lc, j, c] so each w matmul streams the
            # two sequences of a pair as one 512-wide moving operand
            ohT_sb = cp.tile([128, 2, NLC, 2, NCH], b16, name="ohT",
                             tag="ohT")
            for b in range(2):
                for lc in range(NLC):
                    nc.vector.tensor_scalar(
                        ohT_sb[:, b // 2, lc, b % 2, :], iotaC,
                        dTall[:, b * NLC + lc:b * NLC + lc + 1], None,
                        ALU.is_equal,
                    )

            for b in range(2, BPC):
                for lc in range(NLC):
                    nc.vector.tensor_scalar(
                        ohT_sb[:, b // 2, lc, b % 2, :], iotaC,
                        dTall[:, b * NLC + lc:b * NLC + lc + 1], None,
                        ALU.is_equal,
                    )

            # ------------- aT [l, s] = attn.T (unnormalized) ----------
            aT = cp.tile([128, NLC, NS], b16, name="aT", tag="aT")
            for lc in range(NLC):
                tp = pst.tile([128, NS], b16, name=f"tp{lc}", tag="tr",
                              bufs=2)
                nc.tensor.transpose(
                    tp[:], attn[:, lc * 128:(lc + 1) * 128], id32_sb
                )
                if lc % 2 == 0:
                    nc.vector.tensor_copy(aT[:, lc, :], tp[:])
                else:
                    nc.scalar.copy(aT[:, lc, :], tp[:])
            # softmax denominator (only the y eviction consumes it)
            dn = wp.tile([NS, 1], f32, name="dn", tag="dn")
            nc.vector.tensor_tensor(dn[:], dnh[:, 0:1], dnh[:, 1:2], ALU.add)
            rec = wp.tile([NS, 1], f32, name="rec", tag="rec")
            nc.vector.reciprocal(rec[:], dn[:])

            # ------------- y_pe, then per-seq w -> wT (pipelined) -----
            yp = psb.tile([NS, E], f32, name="yp", tag="big")
            for lc in range(NLC):
                nc.tensor.matmul(
                    yp[:], aT[:, lc, :], pe_sb[:, lc, :],
                    start=(lc == 0), stop=False,
                )
            w32_sb = cp.tile([NS, 2, 512], b16, name="w32", tag="w32")
            wT_all = cp.tile([128, 2, NS], b16, name="wT_all", tag="wT_all")
            for pr in range(2):
                wpp = psw.tile([NS, 512], f32, name=f"wp{pr}", tag="wp")
                for lc in range(NLC):
                    nc.tensor.matmul(
                        wpp[:], aT[:, lc, :], ohT_sb[:, pr, lc, :, :],
                        start=(lc == 0), stop=(lc == NLC - 1),
                    )
                for cc in range(4):
                    sl_ = slice(cc * 128, (cc + 1) * 128)
                    if cc % 2 == 0:
                        nc.vector.tensor_copy(w32_sb[:, pr, sl_], wpp[:, sl_])
                    else:
                        nc.scalar.copy(w32_sb[:, pr, sl_], wpp[:, sl_])
                for cc in range(4):
                    b = pr * 2 + cc // 2
                    c = cc % 2
                    tp = pst.tile([128, NS], b16, name=f"wt{pr}_{cc}",
                                  tag="tr", bufs=2)
                    nc.tensor.transpose(
                        tp[:], w32_sb[:, pr, cc * 128:(cc + 1) * 128],
                        id32_sb,
                    )
                    if c == 0:
                        nc.vector.tensor_copy(
                            wT_all[:, c, b * NH:(b + 1) * NH],
                            tp[:, b * NH:(b + 1) * NH])
                    else:
                        nc.scalar.copy(
                            wT_all[:, c, b * NH:(b + 1) * NH],
                            tp[:, b * NH:(b + 1) * NH])

            # ------------- y += wT.T @ emb ; yT ; z = y @ Wv.T --------
            for c in range(2):
                nc.tensor.matmul(
                    yp[:], wT_all[:, c, :], emb_sb[:, c, :],
                    start=False, stop=(c == 1),
                )
            y_sb = wp.tile([NS, E], b16, name="y_sb", tag="y_sb")
            for k in range(4):
                sl_ = slice(k * 128, (k + 1) * 128)
                if k % 2 == 0:
                    nc.vector.tensor_scalar(y_sb[:, sl_], yp[:, sl_],
                                            rec[:], None, ALU.mult)
                else:
                    nc.scalar.activation(y_sb[:, sl_], yp[:, sl_], AF.Copy,
                                         scale=rec[:])
            yT = cp.tile([128, 4, NS], b16, name="yT", tag="yT")
            zp = psb.tile([NS, D], f32, name="zp", tag="big")
            for ech in range(4):
                tp = pst.tile([128, NS], b16, name=f"yt{ech}", tag="tr",
                              bufs=2)
                nc.tensor.transpose(
                    tp[:], y_sb[:, ech * 128:(ech + 1) * 128], id32_sb
                )
                if ech % 2 == 0:
                    nc.vector.tensor_copy(yT[:, ech, :], tp[:])
                else:
                    nc.scalar.copy(yT[:, ech, :], tp[:])
            for ech in range(4):
                nc.tensor.matmul(
                    zp[:], yT[:, ech, :], wvT_sb[:, ech, :],
                    start=(ech == 0), stop=(ech == 3),
                )
            zm = wp.tile([NS, D], b16, name="zm", tag="zm")
            for zh in range(2):
                nc.vector.tensor_tensor(
                    zm[:, zh * 256:(zh + 1) * 256],
                    zp[:, zh * 256:(zh + 1) * 256],
                    hm32_sb[:, zh * 256:(zh + 1) * 256], ALU.mult)
            ctxT = cp.tile([128, 4, BPC], b16, name="ctxT", tag="ctxT")
            for m in range(4):
                p = pst.tile([128, BPC], f32, name=f"cx{m}", tag="tr",
                             bufs=2)
                nc.tensor.matmul(
                    p[:], zm[:, m * 128:(m + 1) * 128], Rsel_sb,
                )
                if m % 2 == 0:
                    nc.vector.tensor_copy(ctxT[:, m, :], p[:])
                else:
                    nc.scalar.copy(ctxT[:, m, :], p[:])

            # ------------- prediction head ----------------------------
            hT = cp.tile([128, 4, BPC], b16, name="hT", tag="hT")
            for hc in range(4):
                p = psq.tile([128, BPC], f32, name=f"hp{hc}", tag="qp")
                for m in range(4):
                    nc.tensor.matmul(
                        p[:], w1T_sb[:, m, hc * 128:(hc + 1) * 128],
                        ctxT[:, m, :], start=(m == 0), stop=(m == 3),
                    )
                t1 = wp.tile([128, BPC], f32, name=f"t1_{hc}", tag="t1",
                             bufs=2)
                nc.vector.tensor_scalar(t1[:], p[:], b1_sb[:, hc:hc + 1],
                                        None, ALU.add)
                nc.vector.scalar_tensor_tensor(
                    hT[:, hc, :], t1[:], 0.01, t1[:], ALU.mult, ALU.max
                )
            r2p = pst.tile([NOUT, BPC], f32, name="r2p", tag="tr", bufs=2)
            for hc in range(4):
                nc.tensor.matmul(
                    r2p[:], w2T_sb[:, hc, :], hT[:, hc, :],
                    start=(hc == 0), stop=(hc == 3),
                )
            r_sb = wp.tile([NOUT, BPC], b16, name="r_sb", tag="r_sb")
            nc.vector.tensor_scalar(r_sb[:], r2p[:], b2_sb, 0.0,
                                    ALU.add, ALU.max)
            mp = pst.tile([1, BPC], f32, name="mp", tag="tr", bufs=2)
            nc.tensor.matmul(mp[:], ones8_sb[:], r_sb[:])
            mt = wp.tile([1, BPC], f32, name="mt", tag="mt")
            nc.vector.tensor_scalar(mt[:], mp[:], 1.0 / NOUT, None, ALU.mult)
            out_sb = cp.tile([1, BPC], f32, name="out_sb", tag="out_sb")
            nc.vector.scalar_tensor_tensor(
                out_sb[:], mt[:], 0.01, mt[:], ALU.mult, ALU.max
            )
            nc.sync.dma_start(out=d_out[:], in_=out_sb[:])

    nc.compile()
    return nc


_CACHE = {}


def _get_module():
    if "nc" not in _CACHE:
        _CACHE["nc"] = _build()
    return _CACHE["nc"]


def _pos_encoding():
    pos = np.arange(LP, dtype=np.float32)[:, None]
    div = np.exp(
        np.arange(0, D, 2, dtype=np.float32) * (-math.log(10000.0) / D)
    )
    pe = np.zeros((LP, D), np.float32)
    pe[:, 0::2] = np.sin(pos * div)
    pe[:, 1::2] = np.cos(pos * div)
    return pe


def make_in_maps(data, lengths, emb, Wq, bq, Wk, bk, Wv, bv, W1, b1, W2, b2):
    # the kernel folds the K-projection into the score lookup; a nonzero
    # bk would add a per-head constant to the scores (bk is zero here).
    assert float(np.abs(np.asarray(bk)).max()) == 0.0
    assert float(np.abs(np.asarray(bv)).max()) == 0.0

    b16 = ml_dtypes.bfloat16
    emb = np.asarray(emb, np.float32)
    Wq, Wk, Wv = (np.asarray(a, np.float32) for a in (Wq, Wk, Wv))
    W1, W2 = np.asarray(W1, np.float32), np.asarray(W2, np.float32)
    pe = _pos_encoding()                          # [LP, D]
    data = np.asarray(data)
    lengths = np.asarray(lengths)
    p = (lengths.astype(np.int64) - 1)

    WqE = emb @ Wq.T                              # [256, 512]
    qpe_full = Wq @ pe[p].T + np.asarray(bq, np.float32)[:, None]  # [D, B]
    hmask = np.repeat(np.eye(NH, dtype=np.float32), DH, axis=0)    # [D, 8]

    cvals = (np.arange(2)[None, :] * 128
             + np.arange(128)[:, None]).astype(np.float32)
    iotaC = np.broadcast_to(np.arange(NCH, dtype=np.float32), (128, NCH))
    hmask4 = hmask.reshape(4, 128, NH).transpose(1, 0, 2).reshape(128, 32)

    E4 = np.zeros((BPC, NS), np.float32)
    for b in range(BPC):
        E4[b, b * NH:(b + 1) * NH] = 1.0
    hm32 = np.zeros((NS, D), np.float32)
    for b in range(BPC):
        for h in range(NH):
            hm32[b * NH + h, h * DH:(h + 1) * DH] = 1.0
    Rsel = np.zeros((NS, BPC), np.float32)
    for b in range(BPC):
        Rsel[b * NH:(b + 1) * NH, b] = 1.0
    id32 = np.eye(NS, dtype=np.float32)

    dpad = np.zeros((B, LP), np.int64)
    dpad[:, :L] = data
    idxl = data[np.arange(B), p].astype(np.float32)

    shared = {
        "wqe": np.ascontiguousarray(WqE, dtype=b16),
        "wk": np.ascontiguousarray(Wk, dtype=b16),
        "embT": np.ascontiguousarray(emb.T, dtype=b16),
        "peT": np.ascontiguousarray(pe.T, dtype=b16),
        "pe": np.ascontiguousarray(pe, dtype=b16),
        "emb": np.ascontiguousarray(emb, dtype=b16),
        "wvT": np.ascontiguousarray(Wv.T, dtype=b16),
        "w1T": np.ascontiguousarray(W1.T, dtype=b16),
        "w2T": np.ascontiguousarray(W2.T, dtype=b16),
    }
    shared["sa"] = np.ascontiguousarray(iotaC, dtype=b16)

    in_maps = []
    for core in range(N_CORES):
        sl = slice(core * BPC, (core + 1) * BPC)
        m = dict(shared)
        dT = np.zeros((128, 32), np.float32)
        for b in range(BPC):
            for lc in range(NLC):
                dT[:, b * NLC + lc] = dpad[sl][b, lc * 128:(lc + 1) * 128]

        maskneg = np.where(
            np.arange(LP)[None, :] > p[sl][:, None], -30000.0, 0.0
        ).astype(np.float32)                       # [4, LP]
        m["m4"] = np.ascontiguousarray(
            np.concatenate([maskneg, E4], axis=1), dtype=b16)
        m["m32b"] = np.ascontiguousarray(
            np.concatenate([hm32, Rsel, id32], axis=1), dtype=b16)

        fb = np.zeros((128, 91), np.float32)
        fb[:, 0:16] = qpe_full[:, sl].reshape(4, 128, BPC).transpose(
            1, 0, 2).reshape(128, 16)
        fb[:, 16:20] = np.asarray(b1, np.float32).reshape(4, 128).T
        fb[0:NOUT, 20] = np.asarray(b2, np.float32)
        fb[:, 21:23] = cvals
        fb[:, 23:55] = dT
        fb[:, 55:87] = hmask4
        fb[:, 87:91] = np.broadcast_to(idxl[sl], (128, BPC))
        m["f32"] = np.ascontiguousarray(fb)

        m["dbb"] = np.ascontiguousarray(np.broadcast_to(
            dpad[sl].astype(np.float32).reshape(1, -1), (128, BPC * LP)
        ).astype(b16))
        in_maps.append(m)
    return in_maps


def kernel(data, lengths, emb, Wq, bq, Wk, bk, Wv, bv, W1, b1, W2, b2):
    nc = _get_module()
    in_maps = make_in_maps(
        np.asarray(data), np.asarray(lengths), emb, Wq, bq, Wk, bk, Wv, bv,
        W1, b1, W2, b2,
    )
    res = run_bass_kernel_spmd(nc, in_maps, list(range(N_CORES)))
    out = np.concatenate(
        [res.results[c]["out"].reshape(BPC) for c in range(N_CORES)]
    )
    return out.astype(np.float32)


# revision 24
# speedup vs baseline: 1.6491x; 1.0511x over previous
"""Trainium2 Bass kernel for nn_Attention_module_52166672777937.

Data-parallel over batch across 8 NeuronCores (4 sequences per core),
with the 4 sequences x 8 heads STACKED on 32 partitions (s=(b,h)) so
every matmul serves all four sequences at once.

Algorithmic restructuring (validated vs the reference in bf16):
  * Only the LAST query row of causal attention is consumed, so scores
    are [32, L] per core, not [B,H,L,L].
  * x = emb[data] + pe is NEVER materialized.  Scores decompose as
      scores[s,l] = s_emb[s, data[l]] + (qk_s . peT[:,l]) + mask
    where s_emb = qkv @ emb.T is a per-head 256-entry lookup table and
    the data lookup is a one-hot matmul.
  * ctx = attn @ x @ Wv.T similarly decomposes:
      y = attn @ x = (attn @ onehot.T) @ emb + attn @ pe.
  * softmax normalization is folded into the attn transposes by using
    diag(1/denominator) as the transpose "identity" matrix.
  * q = Wq(emb[last] + pe[last]) + bq folds to  (emb@Wq.T).T @ onehot_last
    + qpe  with qpe computed host-side from lengths.
"""

import math
import sys

import ml_dtypes
import numpy as np

sys.path.insert(0, "/opt/trn_rl_repo")

import concourse.bacc as bacc
import concourse.bass as bass
import concourse.mybir as mybir
import concourse.tile as tile
from concourse.bass_utils import run_bass_kernel_spmd

dt = mybir.dt
AF = mybir.ActivationFunctionType
ALU = mybir.AluOpType
AX = mybir.AxisListType
PSUM = bass.MemorySpace.PSUM

N_CORES = 8
B, L = 32, 1000
LP = 1024
BPC = B // N_CORES        # 4 sequences per core
NS = BPC * 8              # 32 stacked (seq, head) rows
NCH = 256
E = 512
D = 512
NH, DH = 8, 64
HS = 512
NOUT = 8
SCALE = 1.0 / math.sqrt(DH)
NLC = LP // 128           # 8 position chunks


def _build():
    nc = bacc.Bacc(
        "TRN2", target_bir_lowering=False, debug=False, num_devices=N_CORES
    )

    f32 = dt.float32
    b16 = dt.bfloat16

    # ---- DRAM inputs -------------------------------------------------
    # f32 [128, 55]: qpe | b1 | b2(8 rows) | cvals | dT
    d_f32 = nc.dram_tensor("f32", [128, 91], f32, kind="ExternalInput")
    d_wqe = nc.dram_tensor("wqe", [NCH, D], b16, kind="ExternalInput")
    d_wk = nc.dram_tensor("wk", [D, E], b16, kind="ExternalInput")
    d_embT = nc.dram_tensor("embT", [E, NCH], b16, kind="ExternalInput")
    d_peT = nc.dram_tensor("peT", [E, LP], b16, kind="ExternalInput")
    # [4, 1056]: maskneg | E4 ; [32, 548]: hm32 | Rsel | id32
    d_m4 = nc.dram_tensor("m4", [BPC, 1056], b16, kind="ExternalInput")
    d_m32b = nc.dram_tensor("m32b", [NS, 548], b16, kind="ExternalInput")
    # [128, 288]: iotaC | hmask4
    d_sa = nc.dram_tensor("sa", [128, 256], b16, kind="ExternalInput")
    d_pe = nc.dram_tensor("pe", [LP, E], b16, kind="ExternalInput")
    d_emb = nc.dram_tensor("emb", [NCH, E], b16, kind="ExternalInput")
    d_wvT = nc.dram_tensor("wvT", [E, D], b16, kind="ExternalInput")
    d_w1T = nc.dram_tensor("w1T", [D, HS], b16, kind="ExternalInput")
    d_w2T = nc.dram_tensor("w2T", [HS, NOUT], b16, kind="ExternalInput")
    d_dbb = nc.dram_tensor("dbb", [128, BPC * LP], b16,
                           kind="ExternalInput")
    d_out = nc.dram_tensor("out", [1, BPC], f32, kind="ExternalOutput")

    with tile.TileContext(nc) as tc:
        with (
            tc.tile_pool(name="const", bufs=1) as cp,
            tc.tile_pool(name="work", bufs=2) as wp,
            tc.tile_pool(name="psbig", bufs=2, space=PSUM) as psb,
            tc.tile_pool(name="pstr", bufs=2, space=PSUM) as pst,
            tc.tile_pool(name="psw", bufs=2, space=PSUM) as psw,
            tc.tile_pool(name="psq", bufs=2, space=PSUM) as psq,
        ):
            # ------------- DMA: 3 queues, priority order ---------------
            # sync: f32 -> wqe -> peT halves -> m4 -> m32b
            f32_sb = cp.tile([128, 91], f32, name="f32", tag="f32")
            nc.sync.dma_start(out=f32_sb[:], in_=d_f32[:])
            qpe_sb = f32_sb[:, 0:16]     # [128, 4d x 4b]
            b1_sb = f32_sb[:, 16:20]
            b2_sb = f32_sb[0:NOUT, 20:21]
            cvals = f32_sb[:, 21:23]
            dTall = f32_sb[:, 23:55]
            hmask4 = f32_sb[:, 55:87]
            idxb_sb = f32_sb[:, 87:91]
            wqe_sb = cp.tile([128, 2, D], b16, name="wqe", tag="wqe")
            nc.sync.dma_start(
                out=wqe_sb[:], in_=d_wqe[:].rearrange("(c p) n -> p c n", p=128)
            )
            peT_sb = cp.tile([128, 4, LP], b16, name="peT", tag="peT")
            for hl in range(2):
                nc.sync.dma_start(
                    out=peT_sb[:, :, hl * 512:(hl + 1) * 512],
                    in_=d_peT[:, hl * 512:(hl + 1) * 512].rearrange(
                        "(c p) n -> p c n", p=128),
                )
            m4_sb = cp.tile([BPC, 1056], b16, name="m4", tag="m4")
            nc.sync.dma_start(out=m4_sb[:], in_=d_m4[:])
            maskneg = m4_sb[:, 0:LP]
            E4_sb = m4_sb[:, LP:LP + NS]
            m32b_sb = cp.tile([NS, 548], b16, name="m32b", tag="m32b")
            nc.sync.dma_start(out=m32b_sb[:], in_=d_m32b[:])
            hm32_sb = m32b_sb[:, 0:512]
            Rsel_sb = m32b_sb[:, 512:516]
            id32_sb = m32b_sb[:, 516:548]
            # bulk late-phase weights also on sync: the queue's outstanding
            # limit keeps them from competing with the critical transfers
            pe_sb = cp.tile([128, NLC, E], b16, name="pe", tag="pe")
            nc.sync.dma_start(
                out=pe_sb[:], in_=d_pe[:].rearrange("(c p) n -> p c n", p=128)
            )
            emb_sb = cp.tile([128, 2, E], b16, name="emb", tag="emb")
            nc.sync.dma_start(
                out=emb_sb[:], in_=d_emb[:].rearrange("(c p) n -> p c n", p=128)
            )
            wvT_sb = cp.tile([128, 4, D], b16, name="wvT", tag="wvT")
            nc.sync.dma_start(
                out=wvT_sb[:], in_=d_wvT[:].rearrange("(c p) n -> p c n", p=128)
            )
            w1T_sb = cp.tile([128, 4, HS], b16, name="w1T", tag="w1T")
            nc.sync.dma_start(
                out=w1T_sb[:], in_=d_w1T[:].rearrange("(c p) n -> p c n", p=128)
            )
            w2T_sb = cp.tile([128, 4, NOUT], b16, name="w2T", tag="w2T")
            nc.sync.dma_start(
                out=w2T_sb[:], in_=d_w2T[:].rearrange("(c p) n -> p c n", p=128)
            )
            # scalar: wk -> dbb0 -> embT -> dbb123 -> sa
            wk_sb = cp.tile([128, 4, E], b16, name="wk", tag="wk")
            nc.scalar.dma_start(
                out=wk_sb[:], in_=d_wk[:].rearrange("(c p) n -> p c n", p=128)
            )
            dbb_sb = cp.tile([128, BPC, LP], b16, name="dbb", tag="dbb")
            nc.scalar.dma_start(out=dbb_sb[:, 0, :], in_=d_dbb[:, 0:LP])
            embT_sb = cp.tile([128, 4, NCH], b16, name="embT", tag="embT")
            nc.scalar.dma_start(
                out=embT_sb[:],
                in_=d_embT[:].rearrange("(c p) n -> p c n", p=128),
            )
            for b in range(1, BPC):
                nc.scalar.dma_start(
                    out=dbb_sb[:, b, :],
                    in_=d_dbb[:, b * LP:(b + 1) * LP],
                )
            sa_sb = cp.tile([128, 256], b16, name="sa", tag="sa")
            nc.scalar.dma_start(out=sa_sb[:], in_=d_sa[:])
            iotaC = sa_sb[:, 0:256]
            ones8_sb = cp.tile([NOUT, 1], b16, name="ones8", tag="ones8")
            nc.gpsimd.memset(ones8_sb[:], 1.0)
            s_embm = cp.tile([128, 2, BPC, NS], b16, name="s_embm",
                             tag="s_embm")
            nc.gpsimd.memset(s_embm[:], 0.0)

            # ------------- q prep (emitted first: heads the queues) ---
            ohl = cp.tile([128, 2, BPC], b16, name="ohl", tag="ohl")
            for c in range(2):
                nc.vector.tensor_scalar(
                    ohl[:, c, :], idxb_sb, cvals[:, c:c + 1], None,
                    ALU.is_equal,
                )
            # qT [d, b] = WqE.T @ ohl + qpe   (WqE = emb @ Wq.T)
            qT_sb = cp.tile([128, 4, BPC], f32, name="qT", tag="qT")
            for dch in range(4):
                p = psq.tile([128, BPC], f32, name=f"qp{dch}", tag="qp")
                for c in range(2):
                    nc.tensor.matmul(
                        p[:], wqe_sb[:, c, dch * 128:(dch + 1) * 128],
                        ohl[:, c, :], start=(c == 0), stop=(c == 1),
                    )
                nc.vector.tensor_tensor(
                    qT_sb[:, dch, :], p[:],
                    qpe_sb[:, dch * BPC:(dch + 1) * BPC], ALU.add,
                )
            # qblk [d, s] = q[d, b] * hmask[d, h]
            qblk = cp.tile([128, 4, NS], b16, name="qblk", tag="qblk")
            for dch in range(4):
                for b in range(BPC):
                    nc.vector.tensor_scalar(
                        qblk[:, dch, b * NH:(b + 1) * NH],
                        hmask4[:, dch * NH:(dch + 1) * NH],
                        qT_sb[:, dch, b:b + 1], None, ALU.mult,
                    )
            # qkvT [e, s] = Wk.T @ qblk, scaled by 1/sqrt(dh)
            qkvT = cp.tile([128, 4, NS], b16, name="qkvT", tag="qkvT")
            for ech in range(4):
                p = psq.tile([128, NS], f32, name=f"qkp{ech}", tag="qp")
                for dch in range(4):
                    nc.tensor.matmul(
                        p[:], wk_sb[:, dch, ech * 128:(ech + 1) * 128],
                        qblk[:, dch, :], start=(dch == 0), stop=(dch == 3),
                    )
                nc.vector.tensor_scalar(
                    qkvT[:, ech, :], p[:], SCALE, None, ALU.mult,
                )
            # s_embT [c, s] = embT.T @ qkvT, evicted per-seq masked
            for c in range(2):
                p = psq.tile([128, NS], f32, name=f"sep{c}", tag="qp")
                for ech in range(4):
                    nc.tensor.matmul(
                        p[:], embT_sb[:, ech, c * 128:(c + 1) * 128],
                        qkvT[:, ech, :], start=(ech == 0), stop=(ech == 3),
                    )
                for b in range(BPC):
                    nc.scalar.copy(
                        s_embm[:, c, b, b * NH:(b + 1) * NH],
                        p[:, b * NH:(b + 1) * NH],
                    )

            # ------------- one-hots [c, l] on DVE ---------------------
            oh_sb = cp.tile([128, 2, BPC, LP], b16, name="oh", tag="oh")
            for b in range(BPC):
                for c in range(2):
                    nc.vector.tensor_scalar(
                        oh_sb[:, c, b, :], dbb_sb[:, b, :], cvals[:, c:c + 1],
                        None, ALU.is_equal,
                    )

            # ------------- scores [32, L] + exp -----------------------
            attn = cp.tile([NS, LP], b16, name="attn", tag="attn")
            dnh = wp.tile([NS, 2], f32, name="dnh", tag="dnh")
            for hl in range(2):
                sc = psb.tile([NS, 512], f32, name=f"sc{hl}", tag="big")
                lo, hi = hl * 512, (hl + 1) * 512
                for ech in range(4):
                    nc.tensor.matmul(
                        sc[:], qkvT[:, ech, :], peT_sb[:, ech, lo:hi],
                        start=(ech == 0), stop=False,
                    )
                for b in range(BPC):
                    for c in range(2):
                        nc.tensor.matmul(
                            sc[:], s_embm[:, c, b, :], oh_sb[:, c, b, lo:hi],
                            start=False, stop=False,
                        )
                nc.tensor.matmul(
                    sc[:], E4_sb, maskneg[:, lo:hi], start=False, stop=True,
                )
                nc.scalar.activation(attn[:, lo:hi], sc[:], AF.Exp,
                                     accum_out=dnh[:, hl:hl + 1])

            # ------------- one-hots [l, c] (first half) ---------------
            # ohT layout: [l, pair, lc, j, c] so each w matmul streams the
            # two sequences of a pair as one 512-wide moving operand
            ohT_sb = cp.tile([128, 2, NLC, 2, NCH], b16, name="ohT",
                             tag="ohT")
            for b in range(2):
                for lc in range(NLC):
                    nc.vector.tensor_scalar(
                        ohT_sb[:, b // 2, lc, b % 2, :], iotaC,
                        dTall[:, b * NLC + lc:b * NLC + lc + 1], None,
                        ALU.is_equal,
                    )

            for b in range(2, BPC):
                for lc in range(NLC):
                    nc.vector.tensor_scalar(
                        ohT_sb[:, b // 2, lc, b % 2, :], iotaC,
                        dTall[:, b * NLC + lc:b * NLC + lc + 1], None,
                        ALU.is_equal,
                    )

            # ------------- aT [l, s] = attn.T (unnormalized) ----------
            aT = cp.tile([128, NLC, NS], b16, name="aT", tag="aT")
            for lc in range(NLC):
                tp = pst.tile([128, NS], b16, name=f"tp{lc}", tag="tr",
                              bufs=2)
                nc.tensor.transpose(
                    tp[:], attn[:, lc * 128:(lc + 1) * 128], id32_sb
                )
                if lc % 2 == 0:
                    nc.vector.tensor_copy(aT[:, lc, :], tp[:])
                else:
                    nc.scalar.copy(aT[:, lc, :], tp[:])
            # softmax denominator (only the y eviction consumes it)
            dn = wp.tile([NS, 1], f32, name="dn", tag="dn")
            nc.vector.tensor_tensor(dn[:], dnh[:, 0:1], dnh[:, 1:2], ALU.add)
            rec = wp.tile([NS, 1], f32, name="rec", tag="rec")
            nc.vector.reciprocal(rec[:], dn[:])

            # ------------- y_pe, then per-seq w -> wT (pipelined) -----
            yp = psb.tile([NS, E], f32, name="yp", tag="big")
            for lc in range(NLC):
                nc.tensor.matmul(
                    yp[:], aT[:, lc, :], pe_sb[:, lc, :],
                    start=(lc == 0), stop=False,
                )
            w32_sb = cp.tile([NS, 2, 512], b16, name="w32", tag="w32")
            wT_all = cp.tile([128, 2, NS], b16, name="wT_all", tag="wT_all")
            for pr in range(2):
                wpp = psw.tile([NS, 512], f32, name=f"wp{pr}", tag="wp")
                for lc in range(NLC):
                    nc.tensor.matmul(
                        wpp[:], aT[:, lc, :], ohT_sb[:, pr, lc, :, :],
                        start=(lc == 0), stop=(lc == NLC - 1),
                    )
                for cc in range(4):
                    sl_ = slice(cc * 128, (cc + 1) * 128)
                    if cc % 2 == 0:
                        nc.vector.tensor_copy(w32_sb[:, pr, sl_], wpp[:, sl_])
                    else:
                        nc.scalar.copy(w32_sb[:, pr, sl_], wpp[:, sl_])
                for cc in range(4):
                    b = pr * 2 + cc // 2
                    c = cc % 2
                    tp = pst.tile([128, NS], b16, name=f"wt{pr}_{cc}",
                                  tag="tr", bufs=2)
                    nc.tensor.transpose(
                        tp[:], w32_sb[:, pr, cc * 128:(cc + 1) * 128],
                        id32_sb,
                    )
                    if c == 0:
                        nc.vector.tensor_copy(
                            wT_all[:, c, b * NH:(b + 1) * NH],
                            tp[:, b * NH:(b + 1) * NH])
                    else:
                        nc.scalar.copy(
                            wT_all[:, c, b * NH:(b + 1) * NH],
                            tp[:, b * NH:(b + 1) * NH])

            # ------------- y += wT.T @ emb ; yT ; z = y @ Wv.T --------
            for c in range(2):
                nc.tensor.matmul(
                    yp[:], wT_all[:, c, :], emb_sb[:, c, :],
                    start=False, stop=(c == 1),
                )
            y_sb = wp.tile([NS, E], b16, name="y_sb", tag="y_sb")
            for k in range(4):
                sl_ = slice(k * 128, (k + 1) * 128)
                if k % 2 == 0:
                    nc.vector.tensor_scalar(y_sb[:, sl_], yp[:, sl_],
                                            rec[:], None, ALU.mult)
                else:
                    nc.scalar.activation(y_sb[:, sl_], yp[:, sl_], AF.Copy,
                                         scale=rec[:])
            yT = cp.tile([128, 4, NS], b16, name="yT", tag="yT")
            zp = psb.tile([NS, D], f32, name="zp", tag="big")
            for ech in range(4):
                tp = pst.tile([128, NS], b16, name=f"yt{ech}", tag="tr",
                              bufs=2)
                nc.tensor.transpose(
                    tp[:], y_sb[:, ech * 128:(ech + 1) * 128], id32_sb
                )
                if ech % 2 == 0:
                    nc.vector.tensor_copy(yT[:, ech, :], tp[:])
                else:
                    nc.scalar.copy(yT[:, ech, :], tp[:])
            for ech in range(4):
                nc.tensor.matmul(
                    zp[:], yT[:, ech, :], wvT_sb[:, ech, :],
                    start=(ech == 0), stop=(ech == 3),
                )
            zm = wp.tile([NS, D], b16, name="zm", tag="zm")
            for zh in range(2):
                nc.vector.tensor_tensor(
                    zm[:, zh * 256:(zh + 1) * 256],
                    zp[:, zh * 256:(zh + 1) * 256],
                    hm32_sb[:, zh * 256:(zh + 1) * 256], ALU.mult)
            ctxT = cp.tile([128, 4, BPC], b16, name="ctxT", tag="ctxT")
            for m in range(4):
                p = pst.tile([128, BPC], f32, name=f"cx{m}", tag="tr",
                             bufs=2)
                nc.tensor.matmul(
                    p[:], zm[:, m * 128:(m + 1) * 128], Rsel_sb,
                )
                if m % 2 == 0:
                    nc.vector.tensor_copy(ctxT[:, m, :], p[:])
                else:
                    nc.scalar.copy(ctxT[:, m, :], p[:])

            # ------------- prediction head ----------------------------
            hT = cp.tile([128, 4, BPC], b16, name="hT", tag="hT")
            for hc in range(4):
                p = psq.tile([128, BPC], f32, name=f"hp{hc}", tag="qp")
                for m in range(4):
                    nc.tensor.matmul(
                        p[:], w1T_sb[:, m, hc * 128:(hc + 1) * 128],
                        ctxT[:, m, :], start=(m == 0), stop=(m == 3),
                    )
                t1 = wp.tile([128, BPC], f32, name=f"t1_{hc}", tag="t1",
                             bufs=2)
                nc.vector.tensor_scalar(t1[:], p[:], b1_sb[:, hc:hc + 1],
                                        None, ALU.add)
                nc.vector.scalar_tensor_tensor(
                    hT[:, hc, :], t1[:], 0.01, t1[:], ALU.mult, ALU.max
                )
            r2p = pst.tile([NOUT, BPC], f32, name="r2p", tag="tr", bufs=2)
            for hc in range(4):
                nc.tensor.matmul(
                    r2p[:], w2T_sb[:, hc, :], hT[:, hc, :],
                    start=(hc == 0), stop=(hc == 3),
                )
            r_sb = wp.tile([NOUT, BPC], b16, name="r_sb", tag="r_sb")
            nc.vector.tensor_scalar(r_sb[:], r2p[:], b2_sb, 0.0,
                                    ALU.add, ALU.max)
            mp = pst.tile([1, BPC], f32, name="mp", tag="tr", bufs=2)
            nc.tensor.matmul(mp[:], ones8_sb[:], r_sb[:])
            mt = wp.tile([1, BPC], f32, name="mt", tag="mt")
            nc.vector.tensor_scalar(mt[:], mp[:], 1.0 / NOUT, None, ALU.mult)
            out_sb = cp.tile([1, BPC], f32, name="out_sb", tag="out_sb")
            nc.vector.scalar_tensor_tensor(
                out_sb[:], mt[:], 0.01, mt[:], ALU.mult, ALU.max
            )
            nc.sync.dma_start(out=d_out[:], in_=out_sb[:])

    nc.compile()
    return nc


_CACHE = {}


def _get_module():
    if "nc" not in _CACHE:
        _CACHE["nc"] = _build()
    return _CACHE["nc"]


def _pos_encoding():
    pos = np.arange(LP, dtype=np.float32)[:, None]
    div = np.exp(
        np.arange(0, D, 2, dtype=np.float32) * (-math.log(10000.0) / D)
    )
    pe = np.zeros((LP, D), np.float32)
    pe[:, 0::2] = np.sin(pos * div)
    pe[:, 1::2] = np.cos(pos * div)
    return pe


def make_in_maps(data, lengths, emb, Wq, bq, Wk, bk, Wv, bv, W1, b1, W2, b2):
    # the kernel folds the K-projection into the score lookup; a nonzero
    # bk would add a per-head constant to the scores (bk is zero here).
    assert float(np.abs(np.asarray(bk)).max()) == 0.0
    assert float(np.abs(np.asarray(bv)).max()) == 0.0

    b16 = ml_dtypes.bfloat16
    emb = np.asarray(emb, np.float32)
    Wq, Wk, Wv = (np.asarray(a, np.float32) for a in (Wq, Wk, Wv))
    W1, W2 = np.asarray(W1, np.float32), np.asarray(W2, np.float32)
    pe = _pos_encoding()                          # [LP, D]
    data = np.asarray(data)
    lengths = np.asarray(lengths)
    p = (lengths.astype(np.int64) - 1)

    WqE = emb @ Wq.T                              # [256, 512]
    qpe_full = Wq @ pe[p].T + np.asarray(bq, np.float32)[:, None]  # [D, B]
    hmask = np.repeat(np.eye(NH, dtype=np.float32), DH, axis=0)    # [D, 8]

    cvals = (np.arange(2)[None, :] * 128
             + np.arange(128)[:, None]).astype(np.float32)
    iotaC = np.broadcast_to(np.arange(NCH, dtype=np.float32), (128, NCH))
    hmask4 = hmask.reshape(4, 128, NH).transpose(1, 0, 2).reshape(128, 32)

    E4 = np.zeros((BPC, NS), np.float32)
    for b in range(BPC):
        E4[b, b * NH:(b + 1) * NH] = 1.0
    hm32 = np.zeros((NS, D), np.float32)
    for b in range(BPC):
        for h in range(NH):
            hm32[b * NH + h, h * DH:(h + 1) * DH] = 1.0
    Rsel = np.zeros((NS, BPC), np.float32)
    for b in range(BPC):
        Rsel[b * NH:(b + 1) * NH, b] = 1.0
    id32 = np.eye(NS, dtype=np.float32)

    dpad = np.zeros((B, LP), np.int64)
    dpad[:, :L] = data
    idxl = data[np.arange(B), p].astype(np.float32)

    shared = {
        "wqe": np.ascontiguousarray(WqE, dtype=b16),
        "wk": np.ascontiguousarray(Wk, dtype=b16),
        "embT": np.ascontiguousarray(emb.T, dtype=b16),
        "peT": np.ascontiguousarray(pe.T, dtype=b16),
        "pe": np.ascontiguousarray(pe, dtype=b16),
        "emb": np.ascontiguousarray(emb, dtype=b16),
        "wvT": np.ascontiguousarray(Wv.T, dtype=b16),
        "w1T": np.ascontiguousarray(W1.T, dtype=b16),
        "w2T": np.ascontiguousarray(W2.T, dtype=b16),
    }
    shared["sa"] = np.ascontiguousarray(iotaC, dtype=b16)

    in_maps = []
    for core in range(N_CORES):
        sl = slice(core * BPC, (core + 1) * BPC)
        m = dict(shared)
        dT = np.zeros((128, 32), np.float32)
        for b in range(BPC):
            for lc in range(NLC):
                dT[:, b * NLC + lc] = dpad[sl][b, lc * 128:(lc + 1) * 128]

        maskneg = np.where(
            np.arange(LP)[None, :] > p[sl][:, None], -30000.0, 0.0
        ).astype(np.float32)                       # [4, LP]
        m["m4"] = np.ascontiguousarray(
            np.concatenate([maskneg, E4], axis=1), dtype=b16)
        m["m32b"] = np.ascontiguousarray(
            np.concatenate([hm32, Rsel, id32], axis=1), dtype=b16)

        fb = np.zeros((128, 91), np.float32)
        fb[:, 0:16] = qpe_full[:, sl].reshape(4, 128, BPC).transpose(
            1, 0, 2).reshape(128, 16)
        fb[:, 16:20] = np.asarray(b1, np.float32).reshape(4, 128).T
        fb[0:NOUT, 20] = np.asarray(b2, np.float32)
        fb[:, 21:23] = cvals
        fb[:, 23:55] = dT
        fb[:, 55:87] = hmask4
        fb[:, 87:91] = np.broadcast_to(idxl[sl], (128, BPC))
        m["f32"] = np.ascontiguousarray(fb)

        m["dbb"] = np.ascontiguousarray(np.broadcast_to(
            dpad[sl].astype(np.float32).reshape(1, -1), (128, BPC * LP)
        ).astype(b16))
        in_maps.append(m)
    return in_maps


def kernel(data, lengths, emb, Wq, bq, Wk, bk, Wv, bv, W1, b1, W2, b2):
    nc = _get_module()
    in_maps = make_in_maps(
        np.asarray(data), np.asarray(lengths), emb, Wq, bq, Wk, bk, Wv, bv,
        W1, b1, W2, b2,
    )
    res = run_bass_kernel_spmd(nc, in_maps, list(range(N_CORES)))
    out = np.concatenate(
        [res.results[c]["out"].reshape(BPC) for c in range(N_CORES)]
    )
    return out.astype(np.float32)
